# revision 2
# baseline (speedup 1.0000x reference)
"""LoRA linear on 8 Trainium2 NeuronCores.

out = x @ (W + A @ B)^T + bias
  x: [4, 4096, 4096] f32, W: [4096, 4096], bias: [4096], A: [4096, 16], B: [16, 4096]

Strategy (column-parallel / d_out-sharded):
  - Host: Weff = W + A@B (0.1% of total FLOPs), cast x and Weff to bf16
    (PE runs bf16 at the same 1 cycle/row as fp32r but DMA bytes halve),
    and pre-arrange both as exact SBUF images so every DMA is 128 long
    contiguous per-partition lines.
  - Each core c: out[:, c*512:(c+1)*512] = x @ WeffT[:, c*512:(c+1)*512] + bias_c.
    WeffT shard (4 MB bf16) stays SBUF-resident; x streams in m-blocks on the
    SP HWDGE ring; 32 k-tile matmuls accumulate in PSUM; bias add fused into
    the PSUM->SBUF evacuation on DVE; out stores go out on the ACT HWDGE ring
    so they never stall the x prefetch FIFO.
"""
import numpy as np
import ml_dtypes

import concourse.bacc as bacc
import concourse.mybir as mybir
import concourse.tile as tile
from concourse.bass_utils import run_bass_kernel_spmd

BATCH, SEQ, D = 4, 4096, 4096
M = BATCH * SEQ          # 16384 rows
K = D                    # contraction
N_CORES = 8
OS = D // N_CORES        # 512 output cols per core
KT = K // 128            # 32 k-tiles
MB = 256                 # m-block rows per x stream tile
NB = M // MB             # 64 blocks
XBUFS = 3                # x-block buffering depth

_f32 = mybir.dt.float32
_bf16 = mybir.dt.bfloat16
_bf16_np = ml_dtypes.bfloat16

_COMPILED = None


def _build(repeat=1):
    """repeat>1 wraps the compute in a For_i loop that redundantly recomputes
    the same output -- used only for marginal-cost HW timing (the axon
    dispatch floor is ~80ms, far above the ~1ms kernel)."""
    import contextlib
    nc = bacc.Bacc("TRN2", target_bir_lowering=False, debug=False,
                   num_devices=N_CORES)
    # x pre-blocked on host as exact SBUF tile images: [mb, p, kt*MB + j]
    xT = nc.dram_tensor("xT", [NB, 128, KT * MB], _bf16,
                        kind="ExternalInput").ap()
    # w pre-blocked as one SBUF image: [p, kt*OS + o]
    wT = nc.dram_tensor("wT", [128, KT * OS], _bf16, kind="ExternalInput").ap()
    bias = nc.dram_tensor("bias", [128, OS], _f32, kind="ExternalInput").ap()
    out = nc.dram_tensor("out", [M, OS], _f32, kind="ExternalOutput").ap()

    with tile.TileContext(nc) as tc:
        with tc.tile_pool(name="w", bufs=1) as wp, \
             tc.tile_pool(name="xb", bufs=XBUFS) as xp, \
             tc.tile_pool(name="ob", bufs=4) as op_, \
             tc.tile_pool(name="ps", bufs=4, space="PSUM") as pp:
            w_sb = wp.tile([128, KT * OS], _bf16, tag="w")
            nc.sync.dma_start(out=w_sb[:], in_=wT)
            b_sb = wp.tile([128, OS], _f32, tag="bias")
            nc.sync.dma_start(out=b_sb[:], in_=bias)

            loop_cm = (tc.For_i(0, repeat, 1) if repeat > 1
                       else contextlib.nullcontext())
            with loop_cm:
                for mb in range(NB):
                    xt = xp.tile([128, KT * MB], _bf16, tag="x")
                    # one contiguous 2MB DMA per block (16KB per partition)
                    nc.sync.dma_start(out=xt[:], in_=xT[mb])
                    for ms in range(MB // 128):
                        ps = pp.tile([128, OS], _f32, tag="acc")
                        for kt in range(KT):
                            nc.tensor.matmul(
                                ps[:],
                                xt[:, kt * MB + ms * 128:
                                   kt * MB + ms * 128 + 128],
                                w_sb[:, kt * OS:(kt + 1) * OS],
                                start=(kt == 0), stop=(kt == KT - 1))
                        o_sb = op_.tile([128, OS], _f32, tag="o")
                        nc.vector.tensor_add(o_sb[:], ps[:], b_sb[:])
                        row = mb * MB + ms * 128
                        nc.scalar.dma_start(out=out[row:row + 128, :],
                                            in_=o_sb[:])

    nc.compile()
    return nc


def _compiled():
    global _COMPILED
    if _COMPILED is None:
        _COMPILED = _build()
    return _COMPILED


def _prep_in_maps(x, W, bias, A, B):
    x = np.asarray(x, dtype=np.float32).reshape(M, K)
    W = np.asarray(W, dtype=np.float32)
    bias = np.asarray(bias, dtype=np.float32)
    A = np.asarray(A, dtype=np.float32)
    B = np.asarray(B, dtype=np.float32)

    weff = W + A @ B                              # [D_out, K]
    # x SBUF image per block: [mb, p, kt, j] = x.T[kt*128 + p, mb*MB + j]
    xt = np.ascontiguousarray(x.T).astype(_bf16_np)        # [K, M]
    x_img = np.ascontiguousarray(
        xt.reshape(KT, 128, NB, MB).transpose(2, 1, 0, 3)
    ).reshape(NB, 128, KT * MB)
    # w SBUF image per core: [p, kt, o] = weff.T[kt*128 + p, c*OS + o]
    wt = np.ascontiguousarray(weff.T).astype(_bf16_np)     # [K, D_out]
    wt4 = wt.reshape(KT, 128, D)

    in_maps = []
    for c in range(N_CORES):
        sl = slice(c * OS, (c + 1) * OS)
        w_img = np.ascontiguousarray(
            wt4[:, :, sl].transpose(1, 0, 2)).reshape(128, KT * OS)
        in_maps.append({
            "xT": x_img,
            "wT": w_img,
            "bias": np.tile(bias[sl], (128, 1)),
        })
    return in_maps


def kernel(x, W, bias, A, B):
    nc = _compiled()
    in_maps = _prep_in_maps(x, W, bias, A, B)
    res = run_bass_kernel_spmd(nc, in_maps, core_ids=list(range(N_CORES)),
                               trace=False)
    out = np.concatenate([res.results[c]["out"] for c in range(N_CORES)],
                         axis=1)
    return out.reshape(BATCH, SEQ, D)
